# revision 47
# baseline (speedup 1.0000x reference)
"""Causal self-attention (GPT-2 style) on 8 TRN2 NeuronCores.

Sharding: B=2 x H=12 -> 24 (batch, head) pairs; core c handles batch c//4
and heads [3*(c%4), 3*(c%4)+3). Each core computes QKV for its 3 heads,
causal attention (flash-style, scores^T layout), and a partial output
projection; the host sums the 4 per-batch partials and adds b_proj.

Softmax exp is split between ACT (exact, via the activation affine) and
DVE (Schraudolph bit-trick): scores come out of the PE as A*s
(A = 2^23/ln2 folded into the Q projection scale). A single DVE
tensor_scalar (int16(sp*2^-16 + B*2^-16) with B = (127<<23) - 2^21)
yields the bf16 bit pattern of exp(s); ACT groups compute exact
exp(sp/A) via the free activation scale. Early q-blocks (short softmax
rows) stay exact; elsewhere groups alternate engines, offset per stream
so concurrent groups land on different engines.

Scores are K=64 row-tiled matmul pairs (tile_position (0,0)/(64,0),
heads packed in partition halves) running concurrently on the PE.
The whole kernel is one fused, software-pipelined schedule: QKV(tb)
feeds attention pair J=tb, AV lags S->exp by one group, finalize lags
one pair, and the output projection rides inside the final pairs --
keeping every engine queue free of long waits and the PE clock warm.

Self-contained: builds the Bass program on first call, runs via
run_bass_kernel_spmd on cores 0-7.
"""
import numpy as np
import ml_dtypes

import concourse.bass as bass
import concourse.mybir as mybir
import concourse.tile as tile
from concourse.bass import ts
from concourse.vector_clock import ScopedClock
from concourse.bass_utils import run_bass_kernel_spmd

# ---------------------------------------------------------------------------
# Workaround for the container's walrus build, which rejects any instruction
# carrying more than ONE sync-wait command ("Too many sync wait commands").
# ---------------------------------------------------------------------------
_WAIT_LIMIT = 1


def _patched_drain_and_barrier(self, tick_clock, wait_clock):
    nc = self.nc
    carrier = nc.sync.nop()
    wait_clock.add_sem_waits(carrier.ins, ScopedClock({None: tick_clock.global_clock}))
    si = carrier.ins.sync_info
    waits = list(si.on_wait) if si and si.on_wait else []
    if len(waits) > _WAIT_LIMIT:
        si.on_wait = waits[:_WAIT_LIMIT]
        for w in waits[_WAIT_LIMIT:]:
            n2 = nc.sync.nop()
            s2 = n2.ins.sync_info
            if s2 is None:
                n2.ins.sync_info = mybir.SyncInfo(on_wait=[w], on_update=[])
            else:
                s2.on_wait = [w]
    nc.sync.drain()
    nc.all_engine_barrier()
    popped = nc._tile_sem_poison_stack.pop()
    assert popped is self._sem_poison
    nc.clear_and_free_semaphores(list(self.sems.allocated().values()))
    nc.all_engine_barrier()


tile.TileContext._drain_and_barrier = _patched_drain_and_barrier


def _split_multi_waits(nc):
    n_inserted = 0
    for fn in nc.m.functions:
        for blk in fn.blocks:
            new_list = []
            changed = False
            for inst in blk.instructions:
                si = getattr(inst, "sync_info", None)
                waits = list(si.on_wait) if (si is not None and si.on_wait) else []
                if len(waits) > _WAIT_LIMIT:
                    extra = waits[: len(waits) - _WAIT_LIMIT]
                    keep = waits[len(waits) - _WAIT_LIMIT:]
                    for w in extra:
                        nop = mybir.InstNoOp(
                            name=f"wsplit-{n_inserted}",
                            sync_info=mybir.SyncInfo(on_wait=[w], on_update=[]),
                            bass_nofuse=True,
                            engine=inst.engine,
                        )
                        new_list.append(nop)
                        n_inserted += 1
                    si.on_wait = keep
                    changed = True
                new_list.append(inst)
            if changed:
                blk.instructions = new_list
    return n_inserted


# ---------------------------------------------------------------------------
# Problem constants (hardcoded per contract).
# ---------------------------------------------------------------------------
B, S, E, H = 2, 4096, 768, 12
D = 64           # head dim
HPC = 3          # heads per core
EAUG = 832       # 768 + ones/bias row at 768, zero-padded to 6*128+64
NCORES = 8
BF16 = mybir.dt.bfloat16
F32 = mybir.dt.float32
I16 = mybir.dt.int16
QB = 512         # q-block width (one PSUM bank of fp32)
NQB = S // QB    # 8
NKT = S // 128   # 32 k-tiles

# Schraudolph constants. sp = A*s + B with B = qrow*krow (exact in bf16).
EXP_A = float(2.0 ** 23 / np.log(2.0))
EXP_C = float(2 ** 21)
EXP_B = float((127 << 23) - 2 ** 21)     # 1063256064 = 39936 * 26624
EXP_QROW = 39936.0                       # 39 * 2^10, bf16-exact
EXP_KROW = 26624.0                       # 13 * 2^11, bf16-exact

# exp-engine policy: DVE (Schraudolph) for q-blocks >= DVE_J_MIN on groups
# where g % DVE_G_MOD == 0; ACT (exact) otherwise. Early q-blocks stay
# exact because their short softmax rows don't average the bit-trick error.
DVE_J_MIN = 2
DVE_G_MOD = 2

TRACE = False
LAST_EXEC_NS = None

_nc = {}


def _echunks(with_bias):
    ch = [(e * 128, 128) for e in range(6)]
    if with_bias:
        ch.append((768, 64))  # ones/bias row (+ zero padding)
    return ch


def _build_program(with_bias):
    nc = bass.Bass()
    xT = nc.dram_tensor("xT", [EAUG, S], BF16, kind="ExternalInput")
    wqk = nc.dram_tensor("wqk", [EAUG, 2 * HPC * D], BF16, kind="ExternalInput")
    wv = nc.dram_tensor("wv", [EAUG, HPC * D], BF16, kind="ExternalInput")
    wp = nc.dram_tensor("wp", [2 * 128, E], BF16, kind="ExternalInput")
    tri = nc.dram_tensor("tri", [128, 128], BF16, kind="ExternalInput")
    y = nc.dram_tensor("y", [S, E], F32, kind="ExternalOutput")

    ech = _echunks(with_bias)
    NE = len(ech)

    with tile.TileContext(nc) as tc:
        with (
            tc.tile_pool(name="wpool", bufs=1) as wpool,
            tc.tile_pool(name="per", bufs=1) as per,
        ):
            # --- weights to SBUF ---
            wqk_sb, wv_sb = [], []
            for e, (r0, rn) in enumerate(ech):
                t1 = wpool.tile([rn, 2 * HPC * D], BF16, name=f"wqk{e}")
                nc.sync.dma_start(out=t1, in_=wqk[r0:r0 + rn, :])
                wqk_sb.append(t1)
                t2 = wpool.tile([rn, HPC * D], BF16, name=f"wv{e}")
                nc.sync.dma_start(out=t2, in_=wv[r0:r0 + rn, :])
                wv_sb.append(t2)
            # out-projection weights: wp01 = [wp_h0; wp_h1] (128 rows),
            # wp2 = [wp_h2; zeros]
            wp01_sb = wpool.tile([128, E], BF16, name="wp01")
            nc.sync.dma_start(out=wp01_sb, in_=wp[0:128, :])
            wp2_sb = wpool.tile([128, E], BF16, name="wp2")
            nc.sync.dma_start(out=wp2_sb[0:64, :], in_=wp[128:192, :])
            nc.gpsimd.memset(wp2_sb[64:128, :], 0.0)
            tri_sb = wpool.tile([128, 128], BF16, name="tri_sb")
            nc.sync.dma_start(out=tri_sb, in_=tri[:, :])

            # --- persistent intermediates ---
            # Feature-major Q^T/K^T packed for PE row tiling: heads 0/1 in
            # the two partition halves of qt01/kt01; head 2 duplicated in
            # both halves of qt2/kt2 (so its self-paired streams can row-
            # pack too). Scores matmuls are K=64 with tile_position
            # (0,0)/(64,0) and run pairwise-concurrent on the PE.
            qt01 = per.tile([128, S], BF16, name="qt01")
            kt01 = per.tile([128, S], BF16, name="kt01")
            qt2 = per.tile([128, S], BF16, name="qt2")
            kt2 = per.tile([128, S], BF16, name="kt2")
            # vtok[h]: token-major V with a ones column per k-tile:
            # cols [65i, 65i+64) = V rows, col 65i+64 = 1.0
            vtok = [per.tile([128, 65 * NKT], BF16, name=f"vtok{h}")
                    for h in range(HPC)]
            for h in range(HPC):
                nc.vector.memset(vtok[h], 1.0)
            # normalized O^T: heads 0,1 packed in halves of ot01; head 2 in
            # rows 0:64 of ot2 (rows 64: zero) -> phase 3 runs 2 matmuls.
            ot01 = per.tile([128, S], BF16, name="ot01")
            ot2 = per.tile([128, S], BF16, name="ot2")
            nc.gpsimd.memset(ot2[64:128, :], 0.0)

            # --- fused schedule: QKV(tb) feeds attention pair J=tb ---
            # All PSUM matmul outputs (QKV qkp/vp, scores sp, phase-3 pp)
            # share the 3-slot "sp" tag (2 banks each); the two otp
            # accumulators take the last 2 banks. Interleaving phase 1 into
            # the pair schedule starts ACT exp work at ~7us instead of
            # ~85us, and QKV matmuls fill the PE while exp runs.
            with (
                tc.tile_pool(name="xch", bufs=2) as xch,
                tc.tile_pool(name="asb", bufs=8) as asb,
                tc.tile_pool(name="nrm", bufs=4) as nrm,
                tc.tile_pool(name="sps", bufs=3, space="PSUM") as sps,
                tc.tile_pool(name="ops", bufs=2, space="PSUM") as ops,
                tc.tile_pool(name="psb", bufs=3) as psb,
            ):
                # QKV is emitted as 7 independent units per token block
                # (3 f-slices + 4 V sub-tiles) so it can be fed one unit
                # per attention group as PE filler (smooths PE duty,
                # keeps the clock gate warm). Unit 0 issues the x DMAs.
                qkv_units = []

                def push_qkv(tb):
                    xc = []

                    def dma_x():
                        # ACT's HWDGE queue: runs parallel to the weight
                        # DMAs on the sync queue (kills the startup stall)
                        for e, (r0, rn) in enumerate(ech):
                            t = xch.tile([rn, QB], BF16, name=f"xc{e}",
                                         tag=f"xc{e}")
                            nc.scalar.dma_start(out=t,
                                                in_=xT[r0:r0 + rn, ts(tb, QB)])
                            xc.append(t)

                    # Q^T/K^T: out[f, t] += W[e, f]^T x^T[e, t]
                    # col order [q0|q1|q2|k0|k1|k2] -> f=0 fills qt01
                    # whole, f=1 fills qt2 (duplicated halves) + kt01 lo,
                    # f=2 fills kt01 hi + kt2 (duplicated halves)
                    # Each unit is split into a matmul item and a copy
                    # item pushed separately, so the feed lags the copy
                    # one group behind its matmuls: the ACT/DVE queues
                    # never park waiting on a unit's own PE work.
                    def qk_mm(f, cell):
                        if f == 0:
                            dma_x()
                        ps = sps.tile([128, QB], F32, name="qkp", tag="sp")
                        for e in range(NE):
                            nc.tensor.matmul(ps, wqk_sb[e][:, ts(f, 128)],
                                             xc[e], start=(e == 0),
                                             stop=(e == NE - 1))
                        cell.append(ps)

                    def qk_copy(f, cell):
                        ps = cell.pop()
                        # f=1/2 copies ride on ACT: DVE must stay clear
                        # for the Schraudolph exp cadence
                        tc_ = nc.scalar.copy
                        if f == 0:
                            nc.vector.tensor_copy(qt01[:, ts(tb, QB)], ps)
                        elif f == 1:
                            tc_(qt2[0:64, ts(tb, QB)], ps[0:64, :])
                            tc_(qt2[64:128, ts(tb, QB)], ps[0:64, :])
                            tc_(kt01[0:64, ts(tb, QB)], ps[64:128, :])
                        else:
                            tc_(kt01[64:128, ts(tb, QB)], ps[0:64, :])
                            tc_(kt2[0:64, ts(tb, QB)], ps[64:128, :])
                            tc_(kt2[64:128, ts(tb, QB)], ps[64:128, :])

                    # V token-major: out[t, f] += x^T[e, t]^T W_v[e, f]
                    def v_mm(st, cell):
                        vp = sps.tile([128, HPC * D], F32, name="vp",
                                      tag="sp")
                        for e in range(NE):
                            nc.tensor.matmul(vp, xc[e][:, ts(st, 128)],
                                             wv_sb[e], start=(e == 0),
                                             stop=(e == NE - 1))
                        cell.append(vp)

                    def v_copy(st, cell):
                        vp = cell.pop()
                        kt_idx = 4 * tb + st
                        for h in range(HPC):
                            nc.vector.tensor_copy(
                                vtok[h][:, kt_idx * 65: kt_idx * 65 + 64],
                                vp[:, ts(h, D)])

                    for f in range(3):
                        cell = []
                        qkv_units.append(lambda f=f, c=cell: qk_mm(f, c))
                        qkv_units.append(lambda f=f, c=cell: qk_copy(f, c))
                    for st in range(4):
                        cell = []
                        qkv_units.append(lambda st=st, c=cell: v_mm(st, c))
                        qkv_units.append(lambda st=st, c=cell: v_copy(st, c))

                def drain_qkv(n=None):
                    k = len(qkv_units) if n is None else min(n, len(qkv_units))
                    for _ in range(k):
                        qkv_units.pop(0)()
                def c0_of(J, i):
                    r = i - 4 * J
                    return 0 if r < 0 else 128 * r

                def emit_s(qk, half, J, g, u, sp):
                    i = 2 * g + u
                    c0 = c0_of(J, i)
                    qt, kt = qk
                    p0 = 64 * half
                    # sp^T[k, q] = A*s; K=64 row-tiled, halves run
                    # concurrently on the PE (probe: 1.82x vs serial)
                    nc.tensor.matmul(
                        sp[:, QB * u + c0: QB * (u + 1)],
                        kt[p0:p0 + 64, ts(i, 128)],
                        qt[p0:p0 + 64, QB * J + c0: QB * (J + 1)],
                        start=True, stop=True, tile_position=(p0, 0))

                def emit_exp(J, g, sp, off=0, gmod=DVE_G_MOD):
                    lo = c0_of(J, 2 * g)
                    ex = asb.tile([128, 2 * QB], BF16, name="ex", tag="ex")
                    if J >= DVE_J_MIN and (g + off) % gmod == 0:
                        # Schraudolph: int16(sp*2^-16 + B*2^-16) bits are
                        # the bf16 pattern of exp(s)
                        nc.vector.tensor_scalar(
                            ex.bitcast(I16)[:, lo:], sp[:, lo:],
                            2.0 ** -16, EXP_B / 65536.0,
                            mybir.AluOpType.mult, mybir.AluOpType.add)
                    else:
                        nc.scalar.activation(
                            ex[:, lo:], sp[:, lo:],
                            mybir.ActivationFunctionType.Exp,
                            scale=1.0 / EXP_A)
                    return ex

                def emit_av(h, J, g, otp, ex):
                    imax = 4 * J + 3
                    for u in range(2):
                        i = 2 * g + u
                        r = i - 4 * J
                        c0 = c0_of(J, i)
                        if r >= 0:
                            # zero strictly-future keys in the diagonal
                            # 128x128 sub-block (tri[k,q] = k<=q); GPSIMD
                            # is otherwise idle and this frees DVE for exp
                            nc.gpsimd.tensor_mul(
                                ex[:, QB * u + c0: QB * u + c0 + 128],
                                ex[:, QB * u + c0: QB * u + c0 + 128],
                                tri_sb)
                        # O^T[d, q] (+ row 64 = denominator)
                        nc.tensor.matmul(
                            otp[0:65, c0:QB],
                            vtok[h][:, i * 65:(i + 1) * 65],
                            ex[:, QB * u + c0: QB * (u + 1)],
                            start=(i == 0), stop=(i == imax))

                def finalize(h, J, otp):
                    # 1/den as exp(-ln(den)) on ACT (DVE reciprocal is 8x
                    # slower and stalls the PE long enough to re-throttle).
                    lg = nrm.tile([1, QB], F32, name="lg", tag="lg")
                    nc.scalar.activation(lg, otp[64:65, :],
                                         mybir.ActivationFunctionType.Ln)
                    recb = nrm.tile([1, QB], BF16, name="recb", tag="recb")
                    nc.scalar.activation(recb, lg,
                                         mybir.ActivationFunctionType.Exp,
                                         scale=-1.0)
                    # broadcast 1/denom across 64 partitions via K=1
                    # matmul into rows 64:128 of the SAME otp bank
                    # (tri row 0 = ones); saves a PSUM bank for sps=3.
                    nc.tensor.matmul(otp[64:128, :], tri_sb[0:1, 0:64], recb,
                                     start=True, stop=True)
                    bc = nrm.tile([64, QB], F32, name="bc", tag="bc")
                    nc.vector.tensor_copy(bc, otp[64:128, :])
                    dst = [ot01[0:64], ot01[64:128], ot2[0:64]][h]
                    nc.vector.tensor_mul(dst[:, ts(J, QB)], otp[0:64, :], bc)

                pending_fin = []

                def flush_fin():
                    while pending_fin:
                        pending_fin.pop(0)()

                def run_pair(qk, hA, JA, hB, JB, gmod=DVE_G_MOD,
                             proj_feed=None, early_fin_a=False,
                             post_proj=()):
                    # Software-pipelined: AV consumption lags the S->exp
                    # production by one group, so the in-order PE queue
                    # never parks on an exp wait while the next group's
                    # score matmuls are ready. The previous pair's
                    # finalize chain is emitted after this pair's first
                    # group, when its waits are long resolved.
                    otpA = ops.tile([128, QB], F32, name="otpA", tag="otp")
                    otpB = ops.tile([128, QB], F32, name="otpB", tag="otp")
                    nA, nB = 2 * JA + 2, 2 * JB + 2
                    prev = None
                    for g in range(max(nA, nB) + 1):
                        cur = None
                        if g < max(nA, nB):
                            a = g < nA
                            b = g < nB
                            spA = sps.tile([128, 2 * QB], F32, name="spA",
                                           tag="sp") if a else None
                            spB = sps.tile([128, 2 * QB], F32, name="spB",
                                           tag="sp") if b else None
                            for u in range(2):
                                if a:
                                    emit_s(qk, 0, JA, g, u, spA)
                                if b:
                                    emit_s(qk, 1, JB, g, u, spB)
                            exA = emit_exp(JA, g, spA, 0, gmod) if a else None
                            exB = emit_exp(JB, g, spB, 1, gmod) if b else None
                            cur = (g, exA, exB)
                        if g == 1:
                            flush_fin()
                        if g >= 1 and qkv_units:
                            drain_qkv(1)
                        elif proj_feed is not None and g >= 1:
                            tt = next(proj_feed, None)
                            if tt is not None:
                                emit_proj(tt)
                        if prev is not None:
                            pg, pexA, pexB = prev
                            if pexA is not None:
                                emit_av(hA, JA, pg, otpA, pexA)
                            if pexB is not None:
                                emit_av(hB, JB, pg, otpB, pexB)
                        prev = cur
                        if early_fin_a and g == nA + 1:
                            finalize(hA, JA, otpA)
                            for tt in post_proj:
                                emit_proj(tt)
                    if not early_fin_a:
                        pending_fin.append(lambda: finalize(hA, JA, otpA))
                    pending_fin.append(lambda: finalize(hB, JB, otpB))

                def emit_proj(tt):
                    # phase 3 for token tile tt: y[tt*128:(tt+1)*128, :]
                    y_sb = psb.tile([128, E], F32, name="ysb", tag="ysb")
                    for eh in range(2):
                        pp = sps.tile([128, E // 2], F32, name="pp", tag="sp")
                        nc.tensor.matmul(pp, ot01[:, ts(tt, 128)],
                                         wp01_sb[:, ts(eh, E // 2)],
                                         start=True, stop=False)
                        nc.tensor.matmul(pp, ot2[:, ts(tt, 128)],
                                         wp2_sb[:, ts(eh, E // 2)],
                                         start=False, stop=True)
                        # split the PSUM->SBUF copies across DVE and ACT
                        if eh == 0:
                            nc.vector.tensor_copy(y_sb[:, ts(eh, E // 2)], pp)
                        else:
                            nc.scalar.copy(y_sb[:, ts(eh, E // 2)], pp)
                    nc.sync.dma_start(out=y[ts(tt, 128), :], in_=y_sb)

                # Schedule: QKV(tb) immediately feeds pair J=tb; h2 pairs
                # follow odd tb. The two final pairs have no QKV filler
                # left, so their exp split is pushed to 50/50 DVE/ACT
                # (gmod=2) to shorten the exp-bound causal tail. All
                # phase-3 projection runs at the end.
                # proj tiles 0..23 are fed one-per-group into the two
                # final pairs (their ot inputs are finalized well before,
                # so the copies never park the exp queues); 24..31 need
                # h2's J>=6, finalized only at the very end.
                proj_feed = iter(range(24))
                for tb in range(NQB):
                    # qkv(tb) must be complete before pair J=tb; whatever
                    # the previous pairs' feeds didn't drain goes now.
                    if tb == 0:
                        push_qkv(0)
                    drain_qkv()
                    if tb + 1 < NQB:
                        push_qkv(tb + 1)
                    last = tb == NQB - 1
                    run_pair((qt01, kt01), 0, tb, 1, tb,
                             gmod=2 if last else DVE_G_MOD,
                             proj_feed=proj_feed if last else None)
                    if tb % 2 == 1:
                        run_pair((qt2, kt2), 2, tb - 1, 2, tb,
                                 gmod=2 if last else DVE_G_MOD,
                                 proj_feed=proj_feed if last else None,
                                 early_fin_a=last,
                                 post_proj=range(24, 28) if last else ())
                flush_fin()
                for tt in proj_feed:   # any tiles the feed didn't cover
                    emit_proj(tt)
                for tt in range(28, S // 128):
                    emit_proj(tt)

    _split_multi_waits(nc)
    return nc


def _get_nc(with_bias):
    if with_bias not in _nc:
        _nc[with_bias] = _build_program(with_bias)
    return _nc[with_bias]


def _bf16(a):
    return np.ascontiguousarray(a.astype(ml_dtypes.bfloat16))


def kernel(x, W_attn, b_attn, W_proj, b_proj):
    x = np.asarray(x, dtype=np.float32)
    W_attn = np.asarray(W_attn, dtype=np.float32)
    b_attn = np.asarray(b_attn, dtype=np.float32)
    W_proj = np.asarray(W_proj, dtype=np.float32)
    b_proj = np.asarray(b_proj, dtype=np.float32)

    # q is pre-scaled by A/sqrt(D) so the scores matmul emits A*s (+B via
    # the qt/kt bias rows).
    scale = EXP_A / np.sqrt(np.float32(D))

    # augmented x^T per batch: rows 0..767 = x[b]^T, row 768 = 1, rest 0
    xT_b = []
    for b in range(B):
        xa = np.zeros((EAUG, S), dtype=np.float32)
        xa[:E] = x[b].T
        xa[E] = 1.0
        xT_b.append(_bf16(xa))

    tri_np = _bf16(np.triu(np.ones((128, 128), dtype=np.float32)))

    in_maps = []
    for c in range(NCORES):
        b = c // 4
        heads = [HPC * (c % 4) + j for j in range(HPC)]
        # wqk: [EAUG, 384]; q cols pre-scaled by A/sqrt(D) (bias row too).
        # Column order [q_h0|q_h1|k_h0|k_h1|q_h2|k_h2] so the kernel's
        # f-tiles give each head Q and K at equal base partitions.
        wqk = np.zeros((EAUG, 2 * HPC * D), dtype=np.float32)
        wv = np.zeros((EAUG, HPC * D), dtype=np.float32)
        col_of = {0: 0, 1: 1, 2: 2}          # q column slot per local head
        colk_of = {0: 3, 1: 4, 2: 5}         # k column slot per local head
        for j, h in enumerate(heads):
            wqk[:E, ts_(col_of[j])] = W_attn[:, h * D:(h + 1) * D] * scale
            wqk[E, ts_(col_of[j])] = b_attn[h * D:(h + 1) * D] * scale
            wqk[:E, ts_(colk_of[j])] = W_attn[:, E + h * D:E + (h + 1) * D]
            wqk[E, ts_(colk_of[j])] = b_attn[E + h * D:E + (h + 1) * D]
            wv[:E, ts_(j)] = W_attn[:, 2 * E + h * D:2 * E + (h + 1) * D]
            wv[E, ts_(j)] = b_attn[2 * E + h * D:2 * E + (h + 1) * D]
        # wp dram layout: rows 0..127 = [wp_h0; wp_h1], rows 128..191 = wp_h2
        wpm = np.zeros((2 * 128, E), dtype=np.float32)
        wpm[0:64] = W_proj[heads[0] * D:(heads[0] + 1) * D, :]
        wpm[64:128] = W_proj[heads[1] * D:(heads[1] + 1) * D, :]
        wpm[128:192] = W_proj[heads[2] * D:(heads[2] + 1) * D, :]
        in_maps.append({
            "xT": xT_b[b],
            "wqk": _bf16(wqk),
            "wv": _bf16(wv),
            "wp": _bf16(wpm),
            "tri": tri_np,
        })

    with_bias = bool(np.any(b_attn != 0.0))
    nc = _get_nc(with_bias)
    global LAST_EXEC_NS
    if TRACE:
        _install_ntff_hook()
        res = run_bass_kernel_spmd(nc, in_maps, core_ids=list(range(NCORES)),
                                   trace=True)
        LAST_EXEC_NS = res.exec_time_ns
    else:
        res = run_bass_kernel_spmd(nc, in_maps, core_ids=list(range(NCORES)))

    y = np.zeros((B, S, E), dtype=np.float32)
    for c in range(NCORES):
        y[c // 4] += res.results[c]["y"]
    y += b_proj
    return y


def ts_(j):
    return slice(j * D, (j + 1) * D)


def _install_ntff_hook():
    """Register the axon NTFF profiling hook (dev/profiling only)."""
    import sys, types
    try:
        import antenv
        try:
            from antenv.axon_hooks import get_axon_ntff_profile_hook  # noqa
            return
        except ImportError:
            pass
        hooks_mod = types.ModuleType("antenv.axon_hooks")
        _hook = [None]
        hooks_mod.set_axon_ntff_profile_hook = lambda h: _hook.__setitem__(0, h)
        hooks_mod.get_axon_ntff_profile_hook = lambda: _hook[0]
        sys.modules["antenv.axon_hooks"] = hooks_mod
        antenv.axon_hooks = hooks_mod
        from trn_agent_boot.trn_boot import _ntff_profile_via_ctypes
        hooks_mod.set_axon_ntff_profile_hook(
            _ntff_profile_via_ctypes('/opt/axon/libaxon_pjrt.so'))
    except Exception:
        pass


# revision 48
# speedup vs baseline: 1.0344x; 1.0344x over previous
"""Causal self-attention (GPT-2 style) on 8 TRN2 NeuronCores.

Sharding: B=2 x H=12 -> 24 (batch, head) pairs; core c handles batch c//4
and heads [3*(c%4), 3*(c%4)+3). Each core computes QKV for its 3 heads,
causal attention (flash-style, scores^T layout), and a partial output
projection; the host sums the 4 per-batch partials and adds b_proj.

Softmax exp is split between ACT (exact, via the activation affine) and
DVE (Schraudolph bit-trick): scores come out of the PE as A*s
(A = 2^23/ln2 folded into the Q projection scale). A single DVE
tensor_scalar (int16(sp*2^-16 + B*2^-16) with B = (127<<23) - 2^21)
yields the bf16 bit pattern of exp(s); ACT groups compute exact
exp(sp/A) via the free activation scale. Early q-blocks (short softmax
rows) stay exact; elsewhere groups alternate engines, offset per stream
so concurrent groups land on different engines.

Scores are K=64 row-tiled matmul pairs (tile_position (0,0)/(64,0),
heads packed in partition halves) running concurrently on the PE.
The whole kernel is one fused, software-pipelined schedule: QKV(tb)
feeds attention pair J=tb, AV lags S->exp by one group, finalize lags
one pair, and the output projection rides inside the final pairs --
keeping every engine queue free of long waits and the PE clock warm.

Self-contained: builds the Bass program on first call, runs via
run_bass_kernel_spmd on cores 0-7.
"""
import numpy as np
import ml_dtypes

import concourse.bass as bass
import concourse.mybir as mybir
import concourse.tile as tile
from concourse.bass import ts
from concourse.vector_clock import ScopedClock
from concourse.bass_utils import run_bass_kernel_spmd

# ---------------------------------------------------------------------------
# Workaround for the container's walrus build, which rejects any instruction
# carrying more than ONE sync-wait command ("Too many sync wait commands").
# ---------------------------------------------------------------------------
_WAIT_LIMIT = 1


def _patched_drain_and_barrier(self, tick_clock, wait_clock):
    nc = self.nc
    carrier = nc.sync.nop()
    wait_clock.add_sem_waits(carrier.ins, ScopedClock({None: tick_clock.global_clock}))
    si = carrier.ins.sync_info
    waits = list(si.on_wait) if si and si.on_wait else []
    if len(waits) > _WAIT_LIMIT:
        si.on_wait = waits[:_WAIT_LIMIT]
        for w in waits[_WAIT_LIMIT:]:
            n2 = nc.sync.nop()
            s2 = n2.ins.sync_info
            if s2 is None:
                n2.ins.sync_info = mybir.SyncInfo(on_wait=[w], on_update=[])
            else:
                s2.on_wait = [w]
    nc.sync.drain()
    nc.all_engine_barrier()
    popped = nc._tile_sem_poison_stack.pop()
    assert popped is self._sem_poison
    nc.clear_and_free_semaphores(list(self.sems.allocated().values()))
    nc.all_engine_barrier()


tile.TileContext._drain_and_barrier = _patched_drain_and_barrier


def _split_multi_waits(nc):
    n_inserted = 0
    for fn in nc.m.functions:
        for blk in fn.blocks:
            new_list = []
            changed = False
            for inst in blk.instructions:
                si = getattr(inst, "sync_info", None)
                waits = list(si.on_wait) if (si is not None and si.on_wait) else []
                if len(waits) > _WAIT_LIMIT:
                    extra = waits[: len(waits) - _WAIT_LIMIT]
                    keep = waits[len(waits) - _WAIT_LIMIT:]
                    for w in extra:
                        nop = mybir.InstNoOp(
                            name=f"wsplit-{n_inserted}",
                            sync_info=mybir.SyncInfo(on_wait=[w], on_update=[]),
                            bass_nofuse=True,
                            engine=inst.engine,
                        )
                        new_list.append(nop)
                        n_inserted += 1
                    si.on_wait = keep
                    changed = True
                new_list.append(inst)
            if changed:
                blk.instructions = new_list
    return n_inserted


# ---------------------------------------------------------------------------
# Problem constants (hardcoded per contract).
# ---------------------------------------------------------------------------
B, S, E, H = 2, 4096, 768, 12
D = 64           # head dim
HPC = 3          # heads per core
EAUG = 832       # 768 + ones/bias row at 768, zero-padded to 6*128+64
NCORES = 8
BF16 = mybir.dt.bfloat16
F32 = mybir.dt.float32
I16 = mybir.dt.int16
QB = 512         # q-block width (one PSUM bank of fp32)
NQB = S // QB    # 8
NKT = S // 128   # 32 k-tiles

# Schraudolph constants. sp = A*s + B with B = qrow*krow (exact in bf16).
EXP_A = float(2.0 ** 23 / np.log(2.0))
EXP_C = float(2 ** 21)
EXP_B = float((127 << 23) - 2 ** 21)     # 1063256064 = 39936 * 26624
EXP_QROW = 39936.0                       # 39 * 2^10, bf16-exact
EXP_KROW = 26624.0                       # 13 * 2^11, bf16-exact

# exp-engine policy: DVE (Schraudolph) for q-blocks >= DVE_J_MIN on groups
# where g % DVE_G_MOD == 0; ACT (exact) otherwise. Early q-blocks stay
# exact because their short softmax rows don't average the bit-trick error.
DVE_J_MIN = 2
DVE_G_MOD = 2

TRACE = False
LAST_EXEC_NS = None

_nc = {}


def _echunks(with_bias):
    ch = [(e * 128, 128) for e in range(6)]
    if with_bias:
        ch.append((768, 64))  # ones/bias row (+ zero padding)
    return ch


def _build_program(with_bias):
    nc = bass.Bass()
    xT = nc.dram_tensor("xT", [EAUG, S], BF16, kind="ExternalInput")
    wqk = nc.dram_tensor("wqk", [EAUG, 2 * HPC * D], BF16, kind="ExternalInput")
    wv = nc.dram_tensor("wv", [EAUG, HPC * D], BF16, kind="ExternalInput")
    wp = nc.dram_tensor("wp", [2 * 128, E], BF16, kind="ExternalInput")
    tri = nc.dram_tensor("tri", [128, 128], BF16, kind="ExternalInput")
    y = nc.dram_tensor("y", [S, E], F32, kind="ExternalOutput")

    ech = _echunks(with_bias)
    NE = len(ech)

    with tile.TileContext(nc) as tc:
        with (
            tc.tile_pool(name="wpool", bufs=1) as wpool,
            tc.tile_pool(name="per", bufs=1) as per,
        ):
            # --- weights to SBUF ---
            wqk_sb, wv_sb = [], []
            for e, (r0, rn) in enumerate(ech):
                t1 = wpool.tile([rn, 2 * HPC * D], BF16, name=f"wqk{e}")
                nc.sync.dma_start(out=t1, in_=wqk[r0:r0 + rn, :])
                wqk_sb.append(t1)
                t2 = wpool.tile([rn, HPC * D], BF16, name=f"wv{e}")
                nc.sync.dma_start(out=t2, in_=wv[r0:r0 + rn, :])
                wv_sb.append(t2)
            # out-projection weights: wp01 = [wp_h0; wp_h1] (128 rows),
            # wp2 = [wp_h2; zeros]
            wp01_sb = wpool.tile([128, E], BF16, name="wp01")
            nc.sync.dma_start(out=wp01_sb, in_=wp[0:128, :])
            wp2_sb = wpool.tile([128, E], BF16, name="wp2")
            nc.sync.dma_start(out=wp2_sb[0:64, :], in_=wp[128:192, :])
            nc.gpsimd.memset(wp2_sb[64:128, :], 0.0)
            tri_sb = wpool.tile([128, 128], BF16, name="tri_sb")
            nc.sync.dma_start(out=tri_sb, in_=tri[:, :])

            # --- persistent intermediates ---
            # Feature-major Q^T/K^T packed for PE row tiling: heads 0/1 in
            # the two partition halves of qt01/kt01; head 2 duplicated in
            # both halves of qt2/kt2 (so its self-paired streams can row-
            # pack too). Scores matmuls are K=64 with tile_position
            # (0,0)/(64,0) and run pairwise-concurrent on the PE.
            qt01 = per.tile([128, S], BF16, name="qt01")
            kt01 = per.tile([128, S], BF16, name="kt01")
            qt2 = per.tile([128, S], BF16, name="qt2")
            kt2 = per.tile([128, S], BF16, name="kt2")
            # vtok[h]: token-major V with a ones column per k-tile:
            # cols [65i, 65i+64) = V rows, col 65i+64 = 1.0
            vtok = [per.tile([128, 65 * NKT], BF16, name=f"vtok{h}")
                    for h in range(HPC)]
            for h in range(HPC):
                nc.vector.memset(vtok[h], 1.0)
            # normalized O^T: heads 0,1 packed in halves of ot01; head 2 in
            # rows 0:64 of ot2 (rows 64: zero) -> phase 3 runs 2 matmuls.
            ot01 = per.tile([128, S], BF16, name="ot01")
            ot2 = per.tile([128, S], BF16, name="ot2")
            nc.gpsimd.memset(ot2[64:128, :], 0.0)

            # --- fused schedule: QKV(tb) feeds attention pair J=tb ---
            # All PSUM matmul outputs (QKV qkp/vp, scores sp, phase-3 pp)
            # share the 3-slot "sp" tag (2 banks each); the two otp
            # accumulators take the last 2 banks. Interleaving phase 1 into
            # the pair schedule starts ACT exp work at ~7us instead of
            # ~85us, and QKV matmuls fill the PE while exp runs.
            with (
                tc.tile_pool(name="xch", bufs=2) as xch,
                tc.tile_pool(name="asb", bufs=8) as asb,
                tc.tile_pool(name="nrm", bufs=4) as nrm,
                tc.tile_pool(name="sps", bufs=3, space="PSUM") as sps,
                tc.tile_pool(name="ops", bufs=2, space="PSUM") as ops,
                tc.tile_pool(name="psb", bufs=3) as psb,
            ):
                # QKV is emitted as 7 independent units per token block
                # (3 f-slices + 4 V sub-tiles) so it can be fed one unit
                # per attention group as PE filler (smooths PE duty,
                # keeps the clock gate warm). Unit 0 issues the x DMAs.
                qkv_units = []

                def push_qkv(tb):
                    xc = []

                    def dma_x():
                        # first blocks ride ACT's HWDGE queue, parallel to
                        # the weight DMAs on the sync queue (startup race);
                        # later blocks stay off ACT's busy queue
                        eng = nc.scalar if tb < 2 else nc.sync
                        for e, (r0, rn) in enumerate(ech):
                            t = xch.tile([rn, QB], BF16, name=f"xc{e}",
                                         tag=f"xc{e}")
                            eng.dma_start(out=t,
                                          in_=xT[r0:r0 + rn, ts(tb, QB)])
                            xc.append(t)

                    # Q^T/K^T: out[f, t] += W[e, f]^T x^T[e, t]
                    # col order [q0|q1|q2|k0|k1|k2] -> f=0 fills qt01
                    # whole, f=1 fills qt2 (duplicated halves) + kt01 lo,
                    # f=2 fills kt01 hi + kt2 (duplicated halves)
                    # Each unit is split into a matmul item and a copy
                    # item pushed separately, so the feed lags the copy
                    # one group behind its matmuls: the ACT/DVE queues
                    # never park waiting on a unit's own PE work.
                    def qk_mm(f, cell):
                        if f == 0:
                            dma_x()
                        ps = sps.tile([128, QB], F32, name="qkp", tag="sp")
                        for e in range(NE):
                            nc.tensor.matmul(ps, wqk_sb[e][:, ts(f, 128)],
                                             xc[e], start=(e == 0),
                                             stop=(e == NE - 1))
                        cell.append(ps)

                    def qk_copy(f, cell):
                        ps = cell.pop()
                        # f=1/2 copies ride on ACT: DVE must stay clear
                        # for the Schraudolph exp cadence
                        tc_ = nc.scalar.copy
                        if f == 0:
                            nc.vector.tensor_copy(qt01[:, ts(tb, QB)], ps)
                        elif f == 1:
                            tc_(qt2[0:64, ts(tb, QB)], ps[0:64, :])
                            tc_(qt2[64:128, ts(tb, QB)], ps[0:64, :])
                            tc_(kt01[0:64, ts(tb, QB)], ps[64:128, :])
                        else:
                            tc_(kt01[64:128, ts(tb, QB)], ps[0:64, :])
                            tc_(kt2[0:64, ts(tb, QB)], ps[64:128, :])
                            tc_(kt2[64:128, ts(tb, QB)], ps[64:128, :])

                    # V token-major: out[t, f] += x^T[e, t]^T W_v[e, f]
                    def v_mm(st, cell):
                        vp = sps.tile([128, HPC * D], F32, name="vp",
                                      tag="sp")
                        for e in range(NE):
                            nc.tensor.matmul(vp, xc[e][:, ts(st, 128)],
                                             wv_sb[e], start=(e == 0),
                                             stop=(e == NE - 1))
                        cell.append(vp)

                    def v_copy(st, cell):
                        vp = cell.pop()
                        kt_idx = 4 * tb + st
                        for h in range(HPC):
                            nc.vector.tensor_copy(
                                vtok[h][:, kt_idx * 65: kt_idx * 65 + 64],
                                vp[:, ts(h, D)])

                    for f in range(3):
                        cell = []
                        qkv_units.append(lambda f=f, c=cell: qk_mm(f, c))
                        qkv_units.append(lambda f=f, c=cell: qk_copy(f, c))
                    for st in range(4):
                        cell = []
                        qkv_units.append(lambda st=st, c=cell: v_mm(st, c))
                        qkv_units.append(lambda st=st, c=cell: v_copy(st, c))

                def drain_qkv(n=None):
                    k = len(qkv_units) if n is None else min(n, len(qkv_units))
                    for _ in range(k):
                        qkv_units.pop(0)()
                def c0_of(J, i):
                    r = i - 4 * J
                    return 0 if r < 0 else 128 * r

                def emit_s(qk, half, J, g, u, sp):
                    i = 2 * g + u
                    c0 = c0_of(J, i)
                    qt, kt = qk
                    p0 = 64 * half
                    # sp^T[k, q] = A*s; K=64 row-tiled, halves run
                    # concurrently on the PE (probe: 1.82x vs serial)
                    nc.tensor.matmul(
                        sp[:, QB * u + c0: QB * (u + 1)],
                        kt[p0:p0 + 64, ts(i, 128)],
                        qt[p0:p0 + 64, QB * J + c0: QB * (J + 1)],
                        start=True, stop=True, tile_position=(p0, 0))

                def emit_exp(J, g, sp, off=0, gmod=DVE_G_MOD):
                    lo = c0_of(J, 2 * g)
                    ex = asb.tile([128, 2 * QB], BF16, name="ex", tag="ex")
                    if J >= DVE_J_MIN and (g + off) % gmod == 0:
                        # Schraudolph: int16(sp*2^-16 + B*2^-16) bits are
                        # the bf16 pattern of exp(s)
                        nc.vector.tensor_scalar(
                            ex.bitcast(I16)[:, lo:], sp[:, lo:],
                            2.0 ** -16, EXP_B / 65536.0,
                            mybir.AluOpType.mult, mybir.AluOpType.add)
                    else:
                        nc.scalar.activation(
                            ex[:, lo:], sp[:, lo:],
                            mybir.ActivationFunctionType.Exp,
                            scale=1.0 / EXP_A)
                    return ex

                def emit_av(h, J, g, otp, ex):
                    imax = 4 * J + 3
                    for u in range(2):
                        i = 2 * g + u
                        r = i - 4 * J
                        c0 = c0_of(J, i)
                        if r >= 0:
                            # zero strictly-future keys in the diagonal
                            # 128x128 sub-block (tri[k,q] = k<=q); GPSIMD
                            # is otherwise idle and this frees DVE for exp
                            nc.gpsimd.tensor_mul(
                                ex[:, QB * u + c0: QB * u + c0 + 128],
                                ex[:, QB * u + c0: QB * u + c0 + 128],
                                tri_sb)
                        # O^T[d, q] (+ row 64 = denominator)
                        nc.tensor.matmul(
                            otp[0:65, c0:QB],
                            vtok[h][:, i * 65:(i + 1) * 65],
                            ex[:, QB * u + c0: QB * (u + 1)],
                            start=(i == 0), stop=(i == imax))

                def finalize(h, J, otp):
                    # 1/den as exp(-ln(den)) on ACT (DVE reciprocal is 8x
                    # slower and stalls the PE long enough to re-throttle).
                    lg = nrm.tile([1, QB], F32, name="lg", tag="lg")
                    nc.scalar.activation(lg, otp[64:65, :],
                                         mybir.ActivationFunctionType.Ln)
                    recb = nrm.tile([1, QB], BF16, name="recb", tag="recb")
                    nc.scalar.activation(recb, lg,
                                         mybir.ActivationFunctionType.Exp,
                                         scale=-1.0)
                    # broadcast 1/denom across 64 partitions via K=1
                    # matmul into rows 64:128 of the SAME otp bank
                    # (tri row 0 = ones); saves a PSUM bank for sps=3.
                    nc.tensor.matmul(otp[64:128, :], tri_sb[0:1, 0:64], recb,
                                     start=True, stop=True)
                    bc = nrm.tile([64, QB], F32, name="bc", tag="bc")
                    nc.vector.tensor_copy(bc, otp[64:128, :])
                    dst = [ot01[0:64], ot01[64:128], ot2[0:64]][h]
                    nc.vector.tensor_mul(dst[:, ts(J, QB)], otp[0:64, :], bc)

                pending_fin = []

                def flush_fin():
                    while pending_fin:
                        pending_fin.pop(0)()

                def run_pair(qk, hA, JA, hB, JB, gmod=DVE_G_MOD,
                             proj_feed=None, early_fin_a=False,
                             post_proj=()):
                    # Software-pipelined: AV consumption lags the S->exp
                    # production by one group, so the in-order PE queue
                    # never parks on an exp wait while the next group's
                    # score matmuls are ready. The previous pair's
                    # finalize chain is emitted after this pair's first
                    # group, when its waits are long resolved.
                    otpA = ops.tile([128, QB], F32, name="otpA", tag="otp")
                    otpB = ops.tile([128, QB], F32, name="otpB", tag="otp")
                    nA, nB = 2 * JA + 2, 2 * JB + 2
                    prev = None
                    for g in range(max(nA, nB) + 1):
                        cur = None
                        if g < max(nA, nB):
                            a = g < nA
                            b = g < nB
                            spA = sps.tile([128, 2 * QB], F32, name="spA",
                                           tag="sp") if a else None
                            spB = sps.tile([128, 2 * QB], F32, name="spB",
                                           tag="sp") if b else None
                            for u in range(2):
                                if a:
                                    emit_s(qk, 0, JA, g, u, spA)
                                if b:
                                    emit_s(qk, 1, JB, g, u, spB)
                            exA = emit_exp(JA, g, spA, 0, gmod) if a else None
                            exB = emit_exp(JB, g, spB, 1, gmod) if b else None
                            cur = (g, exA, exB)
                        if g == 1:
                            flush_fin()
                        if g >= 1 and qkv_units:
                            drain_qkv(1)
                        elif proj_feed is not None and g >= 1:
                            tt = next(proj_feed, None)
                            if tt is not None:
                                emit_proj(tt)
                        if prev is not None:
                            pg, pexA, pexB = prev
                            if pexA is not None:
                                emit_av(hA, JA, pg, otpA, pexA)
                            if pexB is not None:
                                emit_av(hB, JB, pg, otpB, pexB)
                        prev = cur
                        if early_fin_a and g == nA + 1:
                            finalize(hA, JA, otpA)
                            for tt in post_proj:
                                emit_proj(tt)
                    if not early_fin_a:
                        pending_fin.append(lambda: finalize(hA, JA, otpA))
                    pending_fin.append(lambda: finalize(hB, JB, otpB))

                def emit_proj(tt):
                    # phase 3 for token tile tt: y[tt*128:(tt+1)*128, :]
                    y_sb = psb.tile([128, E], F32, name="ysb", tag="ysb")
                    for eh in range(2):
                        pp = sps.tile([128, E // 2], F32, name="pp", tag="sp")
                        nc.tensor.matmul(pp, ot01[:, ts(tt, 128)],
                                         wp01_sb[:, ts(eh, E // 2)],
                                         start=True, stop=False)
                        nc.tensor.matmul(pp, ot2[:, ts(tt, 128)],
                                         wp2_sb[:, ts(eh, E // 2)],
                                         start=False, stop=True)
                        # split the PSUM->SBUF copies across DVE and ACT
                        if eh == 0:
                            nc.vector.tensor_copy(y_sb[:, ts(eh, E // 2)], pp)
                        else:
                            nc.scalar.copy(y_sb[:, ts(eh, E // 2)], pp)
                    nc.sync.dma_start(out=y[ts(tt, 128), :], in_=y_sb)

                # Schedule: QKV(tb) immediately feeds pair J=tb; h2 pairs
                # follow odd tb. The two final pairs have no QKV filler
                # left, so their exp split is pushed to 50/50 DVE/ACT
                # (gmod=2) to shorten the exp-bound causal tail. All
                # phase-3 projection runs at the end.
                # proj tiles 0..23 are fed one-per-group into the two
                # final pairs (their ot inputs are finalized well before,
                # so the copies never park the exp queues); 24..31 need
                # h2's J>=6, finalized only at the very end.
                proj_feed = iter(range(24))
                for tb in range(NQB):
                    # qkv(tb) must be complete before pair J=tb; whatever
                    # the previous pairs' feeds didn't drain goes now.
                    if tb == 0:
                        push_qkv(0)
                    drain_qkv()
                    if tb + 1 < NQB:
                        push_qkv(tb + 1)
                    last = tb == NQB - 1
                    run_pair((qt01, kt01), 0, tb, 1, tb,
                             gmod=2 if last else DVE_G_MOD,
                             proj_feed=proj_feed if last else None)
                    if tb % 2 == 1:
                        run_pair((qt2, kt2), 2, tb - 1, 2, tb,
                                 gmod=2 if last else DVE_G_MOD,
                                 proj_feed=proj_feed if last else None,
                                 early_fin_a=last,
                                 post_proj=range(24, 28) if last else ())
                flush_fin()
                for tt in proj_feed:   # any tiles the feed didn't cover
                    emit_proj(tt)
                for tt in range(28, S // 128):
                    emit_proj(tt)

    _split_multi_waits(nc)
    return nc


def _get_nc(with_bias):
    if with_bias not in _nc:
        _nc[with_bias] = _build_program(with_bias)
    return _nc[with_bias]


def _bf16(a):
    return np.ascontiguousarray(a.astype(ml_dtypes.bfloat16))


def kernel(x, W_attn, b_attn, W_proj, b_proj):
    x = np.asarray(x, dtype=np.float32)
    W_attn = np.asarray(W_attn, dtype=np.float32)
    b_attn = np.asarray(b_attn, dtype=np.float32)
    W_proj = np.asarray(W_proj, dtype=np.float32)
    b_proj = np.asarray(b_proj, dtype=np.float32)

    # q is pre-scaled by A/sqrt(D) so the scores matmul emits A*s (+B via
    # the qt/kt bias rows).
    scale = EXP_A / np.sqrt(np.float32(D))

    # augmented x^T per batch: rows 0..767 = x[b]^T, row 768 = 1, rest 0
    xT_b = []
    for b in range(B):
        xa = np.zeros((EAUG, S), dtype=np.float32)
        xa[:E] = x[b].T
        xa[E] = 1.0
        xT_b.append(_bf16(xa))

    tri_np = _bf16(np.triu(np.ones((128, 128), dtype=np.float32)))

    in_maps = []
    for c in range(NCORES):
        b = c // 4
        heads = [HPC * (c % 4) + j for j in range(HPC)]
        # wqk: [EAUG, 384]; q cols pre-scaled by A/sqrt(D) (bias row too).
        # Column order [q_h0|q_h1|k_h0|k_h1|q_h2|k_h2] so the kernel's
        # f-tiles give each head Q and K at equal base partitions.
        wqk = np.zeros((EAUG, 2 * HPC * D), dtype=np.float32)
        wv = np.zeros((EAUG, HPC * D), dtype=np.float32)
        col_of = {0: 0, 1: 1, 2: 2}          # q column slot per local head
        colk_of = {0: 3, 1: 4, 2: 5}         # k column slot per local head
        for j, h in enumerate(heads):
            wqk[:E, ts_(col_of[j])] = W_attn[:, h * D:(h + 1) * D] * scale
            wqk[E, ts_(col_of[j])] = b_attn[h * D:(h + 1) * D] * scale
            wqk[:E, ts_(colk_of[j])] = W_attn[:, E + h * D:E + (h + 1) * D]
            wqk[E, ts_(colk_of[j])] = b_attn[E + h * D:E + (h + 1) * D]
            wv[:E, ts_(j)] = W_attn[:, 2 * E + h * D:2 * E + (h + 1) * D]
            wv[E, ts_(j)] = b_attn[2 * E + h * D:2 * E + (h + 1) * D]
        # wp dram layout: rows 0..127 = [wp_h0; wp_h1], rows 128..191 = wp_h2
        wpm = np.zeros((2 * 128, E), dtype=np.float32)
        wpm[0:64] = W_proj[heads[0] * D:(heads[0] + 1) * D, :]
        wpm[64:128] = W_proj[heads[1] * D:(heads[1] + 1) * D, :]
        wpm[128:192] = W_proj[heads[2] * D:(heads[2] + 1) * D, :]
        in_maps.append({
            "xT": xT_b[b],
            "wqk": _bf16(wqk),
            "wv": _bf16(wv),
            "wp": _bf16(wpm),
            "tri": tri_np,
        })

    with_bias = bool(np.any(b_attn != 0.0))
    nc = _get_nc(with_bias)
    global LAST_EXEC_NS
    if TRACE:
        _install_ntff_hook()
        res = run_bass_kernel_spmd(nc, in_maps, core_ids=list(range(NCORES)),
                                   trace=True)
        LAST_EXEC_NS = res.exec_time_ns
    else:
        res = run_bass_kernel_spmd(nc, in_maps, core_ids=list(range(NCORES)))

    y = np.zeros((B, S, E), dtype=np.float32)
    for c in range(NCORES):
        y[c // 4] += res.results[c]["y"]
    y += b_proj
    return y


def ts_(j):
    return slice(j * D, (j + 1) * D)


def _install_ntff_hook():
    """Register the axon NTFF profiling hook (dev/profiling only)."""
    import sys, types
    try:
        import antenv
        try:
            from antenv.axon_hooks import get_axon_ntff_profile_hook  # noqa
            return
        except ImportError:
            pass
        hooks_mod = types.ModuleType("antenv.axon_hooks")
        _hook = [None]
        hooks_mod.set_axon_ntff_profile_hook = lambda h: _hook.__setitem__(0, h)
        hooks_mod.get_axon_ntff_profile_hook = lambda: _hook[0]
        sys.modules["antenv.axon_hooks"] = hooks_mod
        antenv.axon_hooks = hooks_mod
        from trn_agent_boot.trn_boot import _ntff_profile_via_ctypes
        hooks_mod.set_axon_ntff_profile_hook(
            _ntff_profile_via_ctypes('/opt/axon/libaxon_pjrt.so'))
    except Exception:
        pass
